# revision 34
# baseline (speedup 1.0000x reference)
"""DeepseekV2 MLA attention forward — Trainium2 Bass kernel (8 NeuronCores).

v2: bf16 projections + fp8e4m3 DoubleRow attention + cross-core AllGather.

Sharding: 8 cores = batch(2) x quarter(4). Core (b, c):
  - phase B: kv_a + rmsnorm + k_pe rope for ITS 512-seq quarter -> AllGather#1
  - phase A: q_a + rmsnorm for its 512-query panel (covers AG1)
  - phase D0: kv_b (k_nope^T, V) for ITS 4 heads over full S -> AllGather#2
  - phase C: q_b + q_pe rope for its panel, all 16 heads (covers AG2)
  - phase D: attention for its panel, all heads, fp8 DoubleRow scores/AV
  - phase E: o_proj for its panel
Host only reorders/casts inputs and concatenates output panels.

fp8 score matmul packs the full 192-dim contraction (128 nope + 64 rope)
into one DoubleRow matmul (256-wide contraction, 2x PE rate).
"""

import os
import numpy as np
import ml_dtypes

import concourse.bass as bass
import concourse.bacc as bacc
import concourse.mybir as mybir
import concourse.tile as tile
from concourse import bass_utils

B, S, HID = 2, 2048, 2048
NH = 16
QLR, KVLR = 1536, 512
DN, DR, DV = 128, 64, 128
DQK = DN + DR
SCALE = DQK ** -0.5
EPS = 1e-6
P = 128
W = 512                    # queries per core / seq quarter
NQ = 4                     # quarters per batch
NCORES = 8
NHO = NH // NQ             # own heads per core (4)

F32 = mybir.dt.float32
F32R = mybir.dt.float32r
BF16 = mybir.dt.bfloat16
E4 = mybir.dt.float8e4
EXP = mybir.ActivationFunctionType.Exp
SQRT = mybir.ActivationFunctionType.Sqrt
COPY = mybir.ActivationFunctionType.Copy
MULT = mybir.AluOpType.mult
ADD = mybir.AluOpType.add
DR_MODE = mybir.MatmulPerfMode.DoubleRow

KB_HID = HID // P          # 16
KB_QLR = QLR // P          # 12
KB_CKV = KVLR // P         # 4
KB_S = S // P              # 16
MB_QLR = QLR // P          # 12
MB_NOPE = NH               # 16 blocks of 128 (one per head)
MB_PE = NH // 2            # 8 blocks of 128 (two heads each)
MB_HID = HID // P          # 16
NKT = S // 256             # 8 key tiles of 256 for fp8 attention
GROUPS = [[0, 1, 2, 3], [4, 5, 6, 7]]

LAST_RESULT = None


def _emit(tc, t, with_mask):
    nc = tc.nc

    const = tc.alloc_tile_pool(name="const", bufs=1)
    ones_mat = const.tile([P, P], BF16)
    nc.vector.memset(ones_mat[:], 1.0)
    ones_rowf = const.tile([1, P], F32)
    nc.vector.memset(ones_rowf[:], 1.0)
    ones_rowr = const.tile([1, P], F32R)
    nc.scalar.activation(ones_rowr[:], ones_rowf[:], COPY)
    eps1 = const.tile([1, 1], F32)
    nc.vector.memset(eps1[:], EPS)
    qa_ln = const.tile([P, KB_QLR], F32)
    nc.sync.dma_start(qa_ln[:], t["qa_ln_p"][:])
    kva_ln = const.tile([P, KB_CKV], F32)
    nc.sync.dma_start(kva_ln[:], t["kva_ln_p"][:])

    def rinv_bcast(pool, psum_pool, srow_f32):
        """broadcast [1,n] across partitions via PE, then reciprocal."""
        n = srow_f32.shape[-1]
        ps = psum_pool.tile([P, n], F32, tag="bc")
        nc.tensor.matmul(ps[:], ones_rowr[:], srow_f32, start=True, stop=True)
        rinv = pool.tile([P, n], F32, tag="rinv")
        nc.vector.reciprocal_approx_fast(rinv[:], ps[:])
        return rinv

    qa_pool = tc.alloc_tile_pool(name="qaT", bufs=1)
    qaT = qa_pool.tile([P, KB_QLR, W], BF16)
    wc0 = qa_pool.tile([P, KB_QLR, P], BF16)
    nc.sync.dma_start(wc0[:], t["w_qb_re"][:, 0:P]
                      .rearrange("(k p) c -> p k c", p=P))
    kpe_pool0 = tc.alloc_tile_pool(name="ckkpe", bufs=1)
    kpe2f = kpe_pool0.tile([P, S], BF16)      # roped k_pe dup'd both halves
    ckF = kpe_pool0.tile([P, KB_CKV, S], BF16)  # full normalized ck^T
    hp_pool = tc.alloc_tile_pool(name="hp", bufs=1)
    hp = hp_pool.tile([P, KB_HID, W], BF16)
    nc.sync.dma_start(
        hp[:], t["hsT_rot"][:, 0:W].rearrange("(k p) s -> p k s", p=P))
    waF = hp_pool.tile([P, KB_HID, QLR], BF16)
    nc.sync.dma_start(waF[:], t["w_qa"][:, :]
                      .rearrange("(k p) c -> p k c", p=P))

    # ---------------- phase B: kv_a full S + rmsnorm + kpe rope ------
    with tc.tile_pool(name="phB", bufs=2) as pb, \
         tc.tile_pool(name="phB_w", bufs=1) as pbw, \
         tc.tile_pool(name="phB_ck", bufs=1) as pbc, \
         tc.tile_pool(name="psA", bufs=2, space="PSUM") as psA, \
         tc.tile_pool(name="psS", bufs=2, space="PSUM") as psSS, \
         tc.tile_pool(name="psB", bufs=1, space="PSUM") as psBC:
        wkva = pbw.tile([P, KB_HID, KVLR + P], BF16)
        nc.vector.memset(wkva[:, :, KVLR + DR:], 0.0)
        nc.sync.dma_start(
            wkva[:, :, :KVLR + DR],
            t["w_kva"].rearrange("(k p) c -> p k c", p=P))
        cos1 = pbc.tile([DR, S], F32, tag="cos1")
        nc.sync.dma_start(cos1[:], t["cos1f"][:])
        sin1 = pbc.tile([DR, S], F32, tag="sin1")
        nc.sync.dma_start(sin1[:], t["sin1sf"][:])
        for ch in range(NQ):
            s0 = ch * W
            hch = hp if ch == 0 else None
            if ch > 0:
                hch = pb.tile([P, KB_HID, W], BF16, tag="hch")
                nc.sync.dma_start(
                    hch[:], t["hsT_rot"][:, s0:s0 + W]
                    .rearrange("(k p) s -> p k s", p=P))
            ss = psSS.tile([P, W], F32, tag="ss")
            for m in range(KB_CKV + 1):
                ps = psA.tile([P, W], F32, tag="psA")
                for k in range(KB_HID):
                    nc.tensor.matmul(
                        ps[:], wkva[:, k, m * P:(m + 1) * P], hch[:, k, :],
                        start=(k == 0), stop=(k == KB_HID - 1))
                if m < KB_CKV:
                    nc.scalar.activation(ckF[:, m, s0:s0 + W], ps[:], COPY)
                    sq = pb.tile([P, W], BF16, tag="sq")
                    nc.vector.tensor_tensor(sq[:], ckF[:, m, s0:s0 + W],
                                            ps[:], MULT)
                    nc.tensor.matmul(ss[:], ones_mat[:], sq[:],
                                     start=(m == 0), stop=(m == KB_CKV - 1))
                else:
                    kp = pb.tile([DR, W], BF16, tag="kp")
                    nc.vector.tensor_copy(kp[:], ps[:DR, :])
                    rot = pb.tile([DR, W], BF16, tag="rot")
                    nc.vector.tensor_copy(rot[0:32, :], kp[32:64, :])
                    nc.vector.tensor_copy(rot[32:64, :], kp[0:32, :])
                    tmp = pb.tile([DR, W], BF16, tag="tmpk")
                    nc.vector.tensor_tensor(tmp[:], kp[:],
                                            cos1[:, s0:s0 + W], MULT)
                    nc.vector.tensor_tensor(rot[:], rot[:],
                                            sin1[:, s0:s0 + W], MULT)
                    nc.vector.tensor_tensor(kpe2f[0:DR, s0:s0 + W],
                                            tmp[:], rot[:], ADD)
                    nc.vector.tensor_copy(kpe2f[DR:P, s0:s0 + W],
                                          kpe2f[0:DR, s0:s0 + W])
            srow = pb.tile([1, W], F32R, tag="srow")
            nc.scalar.activation(srow[:], ss[0:1, :], SQRT, bias=eps1[:],
                                 scale=1.0 / KVLR)
            rk = rinv_bcast(pb, psBC, srow[:])
            for m in range(KB_CKV):
                nc.vector.scalar_tensor_tensor(
                    ckF[:, m, s0:s0 + W], ckF[:, m, s0:s0 + W],
                    kva_ln[:, m:m + 1], rk[:], MULT, MULT)

    # ---------------- phase A: q_a panel -> qaT (SBUF, persists) -----
    with tc.tile_pool(name="phA", bufs=2) as pa, \
         tc.tile_pool(name="phA_w", bufs=4) as paw, \
         tc.tile_pool(name="psA", bufs=4, space="PSUM") as psA, \
         tc.tile_pool(name="psS", bufs=1, space="PSUM") as psSS, \
         tc.tile_pool(name="psB", bufs=1, space="PSUM") as psBC:
        ss = psSS.tile([P, W], F32, tag="ss")
        for m in range(MB_QLR):
            ps = psA.tile([P, W], F32, tag="psA")
            for k in range(KB_HID):
                nc.tensor.matmul(ps[:], waF[:, k, m * P:(m + 1) * P],
                                 hp[:, k, :],
                                 start=(k == 0), stop=(k == KB_HID - 1))
            nc.scalar.activation(qaT[:, m, :], ps[:], COPY)
            sq = pa.tile([P, W], BF16, tag="sq")
            nc.vector.tensor_tensor(sq[:], qaT[:, m, :], ps[:], MULT)
            nc.tensor.matmul(ss[:], ones_mat[:], sq[:],
                             start=(m == 0), stop=(m == MB_QLR - 1))
        srow = pa.tile([1, W], F32R, tag="srow")
        nc.scalar.activation(srow[:], ss[0:1, :], SQRT, bias=eps1[:],
                             scale=1.0 / QLR)
        rq = rinv_bcast(pa, psBC, srow[:])
        for m in range(MB_QLR):
            nc.vector.scalar_tensor_tensor(
                qaT[:, m, :], qaT[:, m, :], qa_ln[:, m:m + 1], rq[:],
                MULT, MULT)

    hp_pool.release()

    o_pool = tc.alloc_tile_pool(name="oT", bufs=1)
    oT_sb = o_pool.tile([P, NH, W], BF16)
    q8_pool = tc.alloc_tile_pool(name="q8", bufs=1)
    qnope = q8_pool.tile([P, NH, W], BF16)    # phase C fills
    qpe = q8_pool.tile([P, NH, W], BF16)      # per-head, other 64-half zero
    nc.vector.memset(qpe[:], 0.0)
    kpe_pool = tc.alloc_tile_pool(name="kvw0", bufs=1)
    wv0 = kpe_pool.tile([P, KB_CKV, 2 * DV], BF16)
    nc.sync.dma_start(
        wv0[:], t["w_kvb_re"][:, NH * DN:NH * DN + 2 * DV]
        .rearrange("(k p) c -> p k c", p=P))
    wkn0 = kpe_pool.tile([P, KB_CKV, DN], BF16)
    nc.sync.dma_start(
        wkn0[:], t["w_kvb_re"][:, 0:DN]
        .rearrange("(k p) c -> p k c", p=P))

    # ---------------- phase C: q_b panel (+ q_pe rope) -> q8 ---------
    with tc.tile_pool(name="phC", bufs=2) as pc, \
         tc.tile_pool(name="phC_w", bufs=4) as pcw, \
         tc.tile_pool(name="phC_cs", bufs=1) as pcc, \
         tc.tile_pool(name="psA", bufs=4, space="PSUM") as psA:
        cos2 = pcc.tile([P, W], F32, tag="cos2")
        nc.sync.dma_start(cos2[:], t["cos2p"][:])
        sin2 = pcc.tile([P, W], F32, tag="sin2")
        nc.sync.dma_start(sin2[:], t["sin2sp"][:])
        for m in range(MB_NOPE + MB_PE):
            if m == 0:
                wm = wc0
            else:
                wm = pcw.tile([P, KB_QLR, P], BF16, tag="wqb")
                nc.sync.dma_start(
                    wm[:], t["w_qb_re"][:, m * P:(m + 1) * P]
                    .rearrange("(k p) c -> p k c", p=P))
            ps = psA.tile([P, W], F32, tag="psA")
            for k in range(KB_QLR):
                nc.tensor.matmul(ps[:], wm[:, k, :], qaT[:, k, :],
                                 start=(k == 0), stop=(k == KB_QLR - 1))
            if m < MB_NOPE:
                nc.vector.tensor_copy(qnope[:, m, :], ps[:])
            else:
                j = m - MB_NOPE          # heads 2j, 2j+1 stacked 64+64
                rot = pc.tile([P, W], BF16, tag="rotq")
                for h0 in (0, DR):
                    nc.vector.tensor_copy(rot[h0:h0 + 32, :],
                                          ps[h0 + 32:h0 + 64, :])
                    nc.vector.tensor_copy(rot[h0 + 32:h0 + 64, :],
                                          ps[h0:h0 + 32, :])
                tmp = pc.tile([P, W], BF16, tag="tmpq")
                nc.vector.tensor_tensor(tmp[:], ps[:], cos2[:], MULT)
                nc.vector.tensor_tensor(rot[:], rot[:], sin2[:], MULT)
                nc.vector.tensor_tensor(qpe[0:DR, 2 * j, :],
                                        tmp[0:DR, :], rot[0:DR, :], ADD)
                nc.vector.tensor_tensor(qpe[DR:P, 2 * j + 1, :],
                                        tmp[DR:P, :], rot[DR:P, :], ADD)

    # -------- phase D: per head-pair: V, k_nope, bf16 attention ------
    with tc.tile_pool(name="phD_w", bufs=4) as pdw, \
         tc.tile_pool(name="phD_v", bufs=2) as pdv, \
         tc.tile_pool(name="phD_k", bufs=3) as pdk, \
         tc.tile_pool(name="probs", bufs=6) as pprob, \
         tc.tile_pool(name="phD", bufs=2) as pd, \
         tc.tile_pool(name="psKV", bufs=2, space="PSUM") as psKV, \
         tc.tile_pool(name="psSc", bufs=2, space="PSUM") as psSc, \
         tc.tile_pool(name="psO", bufs=1, space="PSUM") as psO, \
         tc.tile_pool(name="psR", bufs=1, space="PSUM") as psR, \
         tc.tile_pool(name="maskp", bufs=3) as mask_pool:
        def emit_kn_block(h, sc, knt_map):
            if sc == 0:
                if h == 0:
                    wkn = wkn0
                else:
                    wkn = pdw.tile([P, KB_CKV, DN], BF16,
                                   tag=f"wkn{h % 2}")
                    nc.sync.dma_start(
                        wkn[:], t["w_kvb_re"][:, h * DN:(h + 1) * DN]
                        .rearrange("(k p) c -> p k c", p=P))
                knt_map[h] = (pdk.tile([P, S // W, W], BF16, tag="knT",
                                       name=f"knT{h}"), wkn)
            knT, wkn = knt_map[h]
            psk = psKV.tile([P, W], F32, tag="pskv")
            for kc in range(KB_CKV):
                nc.tensor.matmul(
                    psk[:], wkn[:, kc, :],
                    ckF[:, kc, sc * W:(sc + 1) * W],
                    start=(kc == 0), stop=(kc == KB_CKV - 1))
            nc.scalar.activation(knT[:, sc, :], psk[:], COPY)

        knt_map = {}
        for sc in range(S // W):
            emit_kn_block(0, sc, knt_map)
        for g2 in range(NH // 2):
            # V for the two heads of this pair: [keys, 2*128]
            if g2 == 0:
                wv = wv0
            else:
                wv = pdw.tile([P, KB_CKV, 2 * DV], BF16, tag="wv")
                nc.sync.dma_start(
                    wv[:], t["w_kvb_re"][:, NH * DN + g2 * 2 * DV:
                                         NH * DN + (g2 + 1) * 2 * DV]
                    .rearrange("(k p) c -> p k c", p=P))
            v_sb = pdv.tile([P, KB_S, 2 * DV], BF16, tag="v")
            for kb in range(KB_S):
                psv = psKV.tile([P, W], F32, tag="pskv")
                for kc in range(KB_CKV):
                    nc.tensor.matmul(
                        psv[:, :2 * DV],
                        ckF[:, kc, kb * P:(kb + 1) * P], wv[:, kc, :],
                        start=(kc == 0), stop=(kc == KB_CKV - 1))
                nc.scalar.activation(v_sb[:, kb, :], psv[:, :2 * DV], COPY)
            for hl in range(2):
                h = g2 * 2 + hl
                knT = knt_map[h][0]
                po = psO.tile([P, W], F32, tag="po")
                prr = psR.tile([P, W], F32, tag="prr")
                plist = []
                for kt in range(NKT):
                    pss = psSc.tile([P, 2, W], F32, tag="pss")
                    for tt in range(2):
                        kb = 2 * kt + tt
                        sc, j = divmod(kb, 4)
                        nc.tensor.matmul(
                            pss[:, tt, :],
                            knT[:, sc, j * P:(j + 1) * P],
                            qnope[:, h, :], start=True, stop=False)
                        nc.tensor.matmul(
                            pss[:, tt, :],
                            kpe2f[:, kb * P:(kb + 1) * P],
                            qpe[:, h, :], start=False, stop=True)
                    probs = pprob.tile([P, 2, W], BF16, tag="probs")
                    if with_mask:
                        pmf = pd.tile([P, 2, W], F32, tag="pmf")
                        mtile = mask_pool.tile([P, 2, W], F32, tag="mt")
                        nc.sync.dma_start(
                            mtile[:], t["maskT"]
                            .rearrange("(n tp) q -> tp n q", tp=P)
                            [:, 2 * kt:2 * kt + 2, :])
                        nc.vector.scalar_tensor_tensor(
                            pmf[:], pss[:], SCALE, mtile[:], MULT, ADD)
                        nc.scalar.activation(probs[:], pmf[:], EXP)
                    else:
                        nc.scalar.activation(probs[:], pss[:], EXP,
                                             scale=SCALE)
                    plist.append(probs)
                    # next head's k_nope, one full head of lag
                    if kt % 2 == 1 and h + 1 < NH:
                        emit_kn_block(h + 1, (kt - 1) // 2, knt_map)
                    if kt >= 3:
                        _av(nc, v_sb, hl, plist[kt - 3], po, prr,
                            ones_mat, kt - 3)
                for kt in range(NKT - 3, NKT):
                    _av(nc, v_sb, hl, plist[kt], po, prr, ones_mat, kt)
                rec = pd.tile([P, W], F32, tag="rec")
                nc.vector.reciprocal_approx_fast(rec[:], prr[:])
                nc.vector.tensor_tensor(oT_sb[:, h, :], po[:], rec[:], MULT)
                del knt_map[h]
    kpe_pool.release()
    q8_pool.release()

    # ---------------- phase E: o_proj --------------------------------
    with tc.tile_pool(name="phE", bufs=2) as pe, \
         tc.tile_pool(name="phE_w", bufs=2) as pew, \
         tc.tile_pool(name="psA", bufs=2, space="PSUM") as psA:
        for m in range(MB_HID):
            wm = pew.tile([P, NH, P], BF16, tag="wo")
            nc.sync.dma_start(
                wm[:], t["w_o"][:, m * P:(m + 1) * P]
                .rearrange("(k p) c -> p k c", p=P))
            ps = psA.tile([P, W], F32, tag="psA")
            for k in range(NH):
                nc.tensor.matmul(ps[:], wm[:, k, :], oT_sb[:, k, :],
                                 start=(k == 0), stop=(k == NH - 1))
            osb = pe.tile([P, W], F32, tag="osb")
            nc.scalar.activation(osb[:], ps[:], COPY)
            nc.sync.dma_start(t["outT"][m * P:(m + 1) * P, :], osb[:])
    o_pool.release()
    kpe_pool0.release()
    qa_pool.release()
    const.release()


def _av(nc, v_sb, hl, probs, po, prr, ones_mat, kt):
    for tt in range(2):
        kb = 2 * kt + tt
        nc.tensor.matmul(po[:], v_sb[:, kb, hl * DV:(hl + 1) * DV],
                         probs[:, tt, :], start=(kb == 0),
                         stop=(kb == KB_S - 1))
        nc.tensor.matmul(prr[:], ones_mat[:], probs[:, tt, :],
                         start=(kb == 0), stop=(kb == KB_S - 1))


def _build_program(with_mask):
    nc = bacc.Bacc("TRN2", target_bir_lowering=False, debug=False,
                   num_devices=NCORES)
    t = {}

    def inp(name, shape, dt=F32):
        t[name] = nc.dram_tensor(name, list(shape), dt,
                                 kind="ExternalInput").ap()

    inp("hsT_rot", [HID, S], BF16)
    inp("w_qa", [HID, QLR], BF16)
    inp("w_qb_re", [QLR, NH * DQK], BF16)
    inp("w_kva", [HID, KVLR + DR], BF16)
    inp("w_kvb_re", [KVLR, NH * (DN + DV)], BF16)
    inp("w_o", [NH * DV, HID], BF16)
    inp("qa_ln_p", [P, KB_QLR])
    inp("kva_ln_p", [P, KB_CKV])
    inp("cos1f", [DR, S])
    inp("sin1sf", [DR, S])
    inp("cos2p", [P, W])
    inp("sin2sp", [P, W])
    if with_mask:
        inp("maskT", [S, W])
    t["outT"] = nc.dram_tensor("outT", [HID, W], F32,
                               kind="ExternalOutput").ap()

    with tile.TileContext(nc) as tc:
        with nc.allow_low_precision(reason="bf16/fp8 kernel, tol 2e-2"):
            _emit(tc, t, with_mask)
    nc.compile()
    return nc


_PROG_CACHE = {}


def _get_program(with_mask):
    if with_mask not in _PROG_CACHE:
        _PROG_CACHE[with_mask] = _build_program(with_mask)
    return _PROG_CACHE[with_mask]


def make_in_maps(hidden_states, attention_mask, cos, sin, w_qa, qa_ln, w_qb,
                 w_kva, kva_ln, w_kvb, w_o, with_mask):
    f32, bf16 = np.float32, ml_dtypes.bfloat16
    c = np.ascontiguousarray

    w_qb_r = np.asarray(w_qb).reshape(QLR, NH, DQK)
    w_qb_re = c(np.concatenate(
        [w_qb_r[:, :, :DN].reshape(QLR, NH * DN),
         w_qb_r[:, :, DN:].reshape(QLR, NH * DR)], axis=1).astype(bf16))
    w_kvb_r = np.asarray(w_kvb).reshape(KVLR, NH, DN + DV)
    w_kvb_re = c(np.concatenate(
        [w_kvb_r[:, :, :DN].reshape(KVLR, NH * DN),
         w_kvb_r[:, :, DN:].reshape(KVLR, NH * DV)], axis=1).astype(bf16))
    qa_ln_p = c(np.asarray(qa_ln).reshape(KB_QLR, P).T.astype(f32))
    kva_ln_p = c(np.asarray(kva_ln).reshape(KB_CKV, P).T.astype(f32))

    cosT = np.asarray(cos).T.astype(f32)                  # [64, S]
    sinT = np.asarray(sin).T.astype(f32)
    sin_s = np.concatenate([-sinT[:DR // 2], sinT[DR // 2:]], axis=0)
    cos2 = c(np.concatenate([cosT, cosT], axis=0))        # [128, S]
    sin2s = c(np.concatenate([sin_s, sin_s], axis=0))

    shared = {
        "w_qa": c(np.asarray(w_qa).astype(bf16)),
        "w_qb_re": w_qb_re,
        "w_kvb_re": w_kvb_re,
        "w_kva": c(np.asarray(w_kva).astype(bf16)),
        "w_o": c(np.asarray(w_o).astype(bf16)),
        "qa_ln_p": qa_ln_p,
        "kva_ln_p": kva_ln_p,
    }

    hs = np.asarray(hidden_states)
    am = np.asarray(attention_mask)
    in_maps = []
    for core in range(NCORES):
        b, q = divmod(core, NQ)
        q0 = q * W
        m = dict(shared)
        rot = np.r_[np.arange(q0, S), np.arange(0, q0)]
        m["hsT_rot"] = c(hs[b].T[:, rot].astype(bf16))
        m["cos1f"] = c(cosT[:, rot])
        m["sin1sf"] = c(sin_s[:, rot])
        m["cos2p"] = c(cos2[:, q0:q0 + W])
        m["sin2sp"] = c(sin2s[:, q0:q0 + W])
        if with_mask:
            m["maskT"] = c(am[b, 0, q0:q0 + W, :].T[rot, :].astype(f32))
        in_maps.append(m)
    return in_maps


def kernel(hidden_states, attention_mask, cos, sin, w_qa, qa_ln, w_qb,
           w_kva, kva_ln, w_kvb, w_o):
    global LAST_RESULT
    with_mask = bool(np.any(np.asarray(attention_mask) != 0))
    nc = _get_program(with_mask)
    in_maps = make_in_maps(hidden_states, attention_mask, cos, sin, w_qa,
                           qa_ln, w_qb, w_kva, kva_ln, w_kvb, w_o, with_mask)
    trace = os.environ.get("KERNEL_TRACE", "0") == "1"
    res = bass_utils.run_bass_kernel_spmd(
        nc, in_maps, core_ids=list(range(NCORES)), trace=trace)
    LAST_RESULT = res

    out = np.empty((B, S, HID), np.float32)
    for core in range(NCORES):
        b, q = divmod(core, NQ)
        q0 = q * W
        out[b, q0:q0 + W, :] = res.results[core]["outT"].T
    return out


# revision 35
# speedup vs baseline: 1.0214x; 1.0214x over previous
"""DeepseekV2 MLA attention forward — Trainium2 Bass kernel (8 NeuronCores).

v2: bf16 projections + fp8e4m3 DoubleRow attention + cross-core AllGather.

Sharding: 8 cores = batch(2) x quarter(4). Core (b, c):
  - phase B: kv_a + rmsnorm + k_pe rope for ITS 512-seq quarter -> AllGather#1
  - phase A: q_a + rmsnorm for its 512-query panel (covers AG1)
  - phase D0: kv_b (k_nope^T, V) for ITS 4 heads over full S -> AllGather#2
  - phase C: q_b + q_pe rope for its panel, all 16 heads (covers AG2)
  - phase D: attention for its panel, all heads, fp8 DoubleRow scores/AV
  - phase E: o_proj for its panel
Host only reorders/casts inputs and concatenates output panels.

fp8 score matmul packs the full 192-dim contraction (128 nope + 64 rope)
into one DoubleRow matmul (256-wide contraction, 2x PE rate).
"""

import os
import numpy as np
import ml_dtypes

import concourse.bass as bass
import concourse.bacc as bacc
import concourse.mybir as mybir
import concourse.tile as tile
from concourse import bass_utils

B, S, HID = 2, 2048, 2048
NH = 16
QLR, KVLR = 1536, 512
DN, DR, DV = 128, 64, 128
DQK = DN + DR
SCALE = DQK ** -0.5
EPS = 1e-6
P = 128
W = 512                    # queries per core / seq quarter
NQ = 4                     # quarters per batch
NCORES = 8
NHO = NH // NQ             # own heads per core (4)

F32 = mybir.dt.float32
F32R = mybir.dt.float32r
BF16 = mybir.dt.bfloat16
E4 = mybir.dt.float8e4
EXP = mybir.ActivationFunctionType.Exp
SQRT = mybir.ActivationFunctionType.Sqrt
COPY = mybir.ActivationFunctionType.Copy
MULT = mybir.AluOpType.mult
ADD = mybir.AluOpType.add
DR_MODE = mybir.MatmulPerfMode.DoubleRow

KB_HID = HID // P          # 16
KB_QLR = QLR // P          # 12
KB_CKV = KVLR // P         # 4
KB_S = S // P              # 16
MB_QLR = QLR // P          # 12
MB_NOPE = NH               # 16 blocks of 128 (one per head)
MB_PE = NH // 2            # 8 blocks of 128 (two heads each)
MB_HID = HID // P          # 16
NKT = S // 256             # 8 key tiles of 256 for fp8 attention
GROUPS = [[0, 1, 2, 3], [4, 5, 6, 7]]

LAST_RESULT = None


def _emit(tc, t, with_mask):
    nc = tc.nc

    const = tc.alloc_tile_pool(name="const", bufs=1)
    ones_mat = const.tile([P, P], BF16)
    nc.vector.memset(ones_mat[:], 1.0)
    ones_rowf = const.tile([1, P], F32)
    nc.vector.memset(ones_rowf[:], 1.0)
    ones_rowr = const.tile([1, P], F32R)
    nc.scalar.activation(ones_rowr[:], ones_rowf[:], COPY)
    eps1 = const.tile([1, 1], F32)
    nc.vector.memset(eps1[:], EPS)
    qa_ln = const.tile([P, KB_QLR], F32)
    nc.sync.dma_start(qa_ln[:], t["qa_ln_p"][:])
    kva_ln = const.tile([P, KB_CKV], F32)
    nc.sync.dma_start(kva_ln[:], t["kva_ln_p"][:])

    def rinv_bcast(pool, psum_pool, srow_f32):
        """broadcast [1,n] across partitions via PE, then reciprocal."""
        n = srow_f32.shape[-1]
        ps = psum_pool.tile([P, n], F32, tag="bc")
        nc.tensor.matmul(ps[:], ones_rowr[:], srow_f32, start=True, stop=True)
        rinv = pool.tile([P, n], F32, tag="rinv")
        nc.vector.reciprocal_approx_fast(rinv[:], ps[:])
        return rinv

    qa_pool = tc.alloc_tile_pool(name="qaT", bufs=1)
    qaT = qa_pool.tile([P, KB_QLR, W], BF16)
    wc0 = qa_pool.tile([P, KB_QLR, P], BF16)
    kpe_pool0 = tc.alloc_tile_pool(name="ckkpe", bufs=1)
    kpe2f = kpe_pool0.tile([P, S], BF16)      # roped k_pe dup'd both halves
    ckF = kpe_pool0.tile([P, KB_CKV, S], BF16)  # full normalized ck^T
    hp_pool = tc.alloc_tile_pool(name="hp", bufs=1)
    hp = hp_pool.tile([P, KB_HID, W], BF16)
    nc.sync.dma_start(
        hp[:], t["hsT_rot"][:, 0:W].rearrange("(k p) s -> p k s", p=P))
    waF = hp_pool.tile([P, KB_HID, QLR], BF16)

    # ---------------- phase B: kv_a full S + rmsnorm + kpe rope ------
    with tc.tile_pool(name="phB", bufs=2) as pb, \
         tc.tile_pool(name="phB_w", bufs=1) as pbw, \
         tc.tile_pool(name="phB_ck", bufs=1) as pbc, \
         tc.tile_pool(name="psA", bufs=2, space="PSUM") as psA, \
         tc.tile_pool(name="psS", bufs=2, space="PSUM") as psSS, \
         tc.tile_pool(name="psB", bufs=1, space="PSUM") as psBC:
        wkva = pbw.tile([P, KB_HID, KVLR + P], BF16)
        nc.vector.memset(wkva[:, :, KVLR + DR:], 0.0)
        nc.sync.dma_start(
            wkva[:, :, :KVLR + DR],
            t["w_kva"].rearrange("(k p) c -> p k c", p=P))
        cos1 = pbc.tile([DR, S], F32, tag="cos1")
        nc.sync.dma_start(cos1[:], t["cos1f"][:])
        sin1 = pbc.tile([DR, S], F32, tag="sin1")
        nc.sync.dma_start(sin1[:], t["sin1sf"][:])
        for ch in range(NQ):
            s0 = ch * W
            hch = hp if ch == 0 else None
            if ch > 0:
                hch = pb.tile([P, KB_HID, W], BF16, tag="hch")
                nc.sync.dma_start(
                    hch[:], t["hsT_rot"][:, s0:s0 + W]
                    .rearrange("(k p) s -> p k s", p=P))
            ss = psSS.tile([P, W], F32, tag="ss")
            for m in range(KB_CKV + 1):
                ps = psA.tile([P, W], F32, tag="psA")
                for k in range(KB_HID):
                    nc.tensor.matmul(
                        ps[:], wkva[:, k, m * P:(m + 1) * P], hch[:, k, :],
                        start=(k == 0), stop=(k == KB_HID - 1))
                if m < KB_CKV:
                    nc.scalar.activation(ckF[:, m, s0:s0 + W], ps[:], COPY)
                    sq = pb.tile([P, W], BF16, tag="sq")
                    nc.vector.tensor_tensor(sq[:], ckF[:, m, s0:s0 + W],
                                            ps[:], MULT)
                    nc.tensor.matmul(ss[:], ones_mat[:], sq[:],
                                     start=(m == 0), stop=(m == KB_CKV - 1))
                else:
                    kp = pb.tile([DR, W], BF16, tag="kp")
                    nc.vector.tensor_copy(kp[:], ps[:DR, :])
                    rot = pb.tile([DR, W], BF16, tag="rot")
                    nc.vector.tensor_copy(rot[0:32, :], kp[32:64, :])
                    nc.vector.tensor_copy(rot[32:64, :], kp[0:32, :])
                    tmp = pb.tile([DR, W], BF16, tag="tmpk")
                    nc.vector.tensor_tensor(tmp[:], kp[:],
                                            cos1[:, s0:s0 + W], MULT)
                    nc.vector.tensor_tensor(rot[:], rot[:],
                                            sin1[:, s0:s0 + W], MULT)
                    nc.vector.tensor_tensor(kpe2f[0:DR, s0:s0 + W],
                                            tmp[:], rot[:], ADD)
                    nc.vector.tensor_copy(kpe2f[DR:P, s0:s0 + W],
                                          kpe2f[0:DR, s0:s0 + W])
            srow = pb.tile([1, W], F32R, tag="srow")
            nc.scalar.activation(srow[:], ss[0:1, :], SQRT, bias=eps1[:],
                                 scale=1.0 / KVLR)
            rk = rinv_bcast(pb, psBC, srow[:])
            for m in range(KB_CKV):
                nc.vector.scalar_tensor_tensor(
                    ckF[:, m, s0:s0 + W], ckF[:, m, s0:s0 + W],
                    kva_ln[:, m:m + 1], rk[:], MULT, MULT)
            # deferred preloads for phases A/C, behind B's critical DMAs
            if ch == 0:
                nc.sync.dma_start(wc0[:], t["w_qb_re"][:, 0:P]
                                  .rearrange("(k p) c -> p k c", p=P))
                nc.sync.dma_start(
                    waF[:, :, 0:QLR // 2],
                    t["w_qa"][:, 0:QLR // 2]
                    .rearrange("(k p) c -> p k c", p=P))
            elif ch == 1:
                nc.sync.dma_start(
                    waF[:, :, QLR // 2:],
                    t["w_qa"][:, QLR // 2:]
                    .rearrange("(k p) c -> p k c", p=P))

    # ---------------- phase A: q_a panel -> qaT (SBUF, persists) -----
    with tc.tile_pool(name="phA", bufs=2) as pa, \
         tc.tile_pool(name="phA_w", bufs=4) as paw, \
         tc.tile_pool(name="psA", bufs=4, space="PSUM") as psA, \
         tc.tile_pool(name="psS", bufs=1, space="PSUM") as psSS, \
         tc.tile_pool(name="psB", bufs=1, space="PSUM") as psBC:
        ss = psSS.tile([P, W], F32, tag="ss")
        for m in range(MB_QLR):
            ps = psA.tile([P, W], F32, tag="psA")
            for k in range(KB_HID):
                nc.tensor.matmul(ps[:], waF[:, k, m * P:(m + 1) * P],
                                 hp[:, k, :],
                                 start=(k == 0), stop=(k == KB_HID - 1))
            nc.scalar.activation(qaT[:, m, :], ps[:], COPY)
            sq = pa.tile([P, W], BF16, tag="sq")
            nc.vector.tensor_tensor(sq[:], qaT[:, m, :], ps[:], MULT)
            nc.tensor.matmul(ss[:], ones_mat[:], sq[:],
                             start=(m == 0), stop=(m == MB_QLR - 1))
        srow = pa.tile([1, W], F32R, tag="srow")
        nc.scalar.activation(srow[:], ss[0:1, :], SQRT, bias=eps1[:],
                             scale=1.0 / QLR)
        rq = rinv_bcast(pa, psBC, srow[:])
        for m in range(MB_QLR):
            nc.vector.scalar_tensor_tensor(
                qaT[:, m, :], qaT[:, m, :], qa_ln[:, m:m + 1], rq[:],
                MULT, MULT)

    hp_pool.release()

    o_pool = tc.alloc_tile_pool(name="oT", bufs=1)
    oT_sb = o_pool.tile([P, NH, W], BF16)
    q8_pool = tc.alloc_tile_pool(name="q8", bufs=1)
    qnope = q8_pool.tile([P, NH, W], BF16)    # phase C fills
    qpe = q8_pool.tile([P, NH, W], BF16)      # per-head, other 64-half zero
    nc.vector.memset(qpe[:], 0.0)
    kpe_pool = tc.alloc_tile_pool(name="kvw0", bufs=1)
    wv0 = kpe_pool.tile([P, KB_CKV, 2 * DV], BF16)
    nc.sync.dma_start(
        wv0[:], t["w_kvb_re"][:, NH * DN:NH * DN + 2 * DV]
        .rearrange("(k p) c -> p k c", p=P))
    wkn0 = kpe_pool.tile([P, KB_CKV, DN], BF16)
    nc.sync.dma_start(
        wkn0[:], t["w_kvb_re"][:, 0:DN]
        .rearrange("(k p) c -> p k c", p=P))

    # ---------------- phase C: q_b panel (+ q_pe rope) -> q8 ---------
    with tc.tile_pool(name="phC", bufs=2) as pc, \
         tc.tile_pool(name="phC_w", bufs=4) as pcw, \
         tc.tile_pool(name="phC_cs", bufs=1) as pcc, \
         tc.tile_pool(name="psA", bufs=4, space="PSUM") as psA:
        cos2 = pcc.tile([P, W], F32, tag="cos2")
        nc.sync.dma_start(cos2[:], t["cos2p"][:])
        sin2 = pcc.tile([P, W], F32, tag="sin2")
        nc.sync.dma_start(sin2[:], t["sin2sp"][:])
        for m in range(MB_NOPE + MB_PE):
            if m == 0:
                wm = wc0
            else:
                wm = pcw.tile([P, KB_QLR, P], BF16, tag="wqb")
                nc.sync.dma_start(
                    wm[:], t["w_qb_re"][:, m * P:(m + 1) * P]
                    .rearrange("(k p) c -> p k c", p=P))
            ps = psA.tile([P, W], F32, tag="psA")
            for k in range(KB_QLR):
                nc.tensor.matmul(ps[:], wm[:, k, :], qaT[:, k, :],
                                 start=(k == 0), stop=(k == KB_QLR - 1))
            if m < MB_NOPE:
                nc.vector.tensor_copy(qnope[:, m, :], ps[:])
            else:
                j = m - MB_NOPE          # heads 2j, 2j+1 stacked 64+64
                rot = pc.tile([P, W], BF16, tag="rotq")
                for h0 in (0, DR):
                    nc.vector.tensor_copy(rot[h0:h0 + 32, :],
                                          ps[h0 + 32:h0 + 64, :])
                    nc.vector.tensor_copy(rot[h0 + 32:h0 + 64, :],
                                          ps[h0:h0 + 32, :])
                tmp = pc.tile([P, W], BF16, tag="tmpq")
                nc.vector.tensor_tensor(tmp[:], ps[:], cos2[:], MULT)
                nc.vector.tensor_tensor(rot[:], rot[:], sin2[:], MULT)
                nc.vector.tensor_tensor(qpe[0:DR, 2 * j, :],
                                        tmp[0:DR, :], rot[0:DR, :], ADD)
                nc.vector.tensor_tensor(qpe[DR:P, 2 * j + 1, :],
                                        tmp[DR:P, :], rot[DR:P, :], ADD)

    # -------- phase D: per head-pair: V, k_nope, bf16 attention ------
    with tc.tile_pool(name="phD_w", bufs=4) as pdw, \
         tc.tile_pool(name="phD_v", bufs=2) as pdv, \
         tc.tile_pool(name="phD_k", bufs=3) as pdk, \
         tc.tile_pool(name="probs", bufs=6) as pprob, \
         tc.tile_pool(name="phD", bufs=2) as pd, \
         tc.tile_pool(name="psKV", bufs=2, space="PSUM") as psKV, \
         tc.tile_pool(name="psSc", bufs=2, space="PSUM") as psSc, \
         tc.tile_pool(name="psO", bufs=1, space="PSUM") as psO, \
         tc.tile_pool(name="psR", bufs=1, space="PSUM") as psR, \
         tc.tile_pool(name="maskp", bufs=3) as mask_pool:
        def emit_kn_block(h, sc, knt_map):
            if sc == 0:
                if h == 0:
                    wkn = wkn0
                else:
                    wkn = pdw.tile([P, KB_CKV, DN], BF16,
                                   tag=f"wkn{h % 2}")
                    nc.sync.dma_start(
                        wkn[:], t["w_kvb_re"][:, h * DN:(h + 1) * DN]
                        .rearrange("(k p) c -> p k c", p=P))
                knt_map[h] = (pdk.tile([P, S // W, W], BF16, tag="knT",
                                       name=f"knT{h}"), wkn)
            knT, wkn = knt_map[h]
            psk = psKV.tile([P, W], F32, tag="pskv")
            for kc in range(KB_CKV):
                nc.tensor.matmul(
                    psk[:], wkn[:, kc, :],
                    ckF[:, kc, sc * W:(sc + 1) * W],
                    start=(kc == 0), stop=(kc == KB_CKV - 1))
            nc.scalar.activation(knT[:, sc, :], psk[:], COPY)

        knt_map = {}
        for sc in range(S // W):
            emit_kn_block(0, sc, knt_map)
        for g2 in range(NH // 2):
            # V for the two heads of this pair: [keys, 2*128]
            if g2 == 0:
                wv = wv0
            else:
                wv = pdw.tile([P, KB_CKV, 2 * DV], BF16, tag="wv")
                nc.sync.dma_start(
                    wv[:], t["w_kvb_re"][:, NH * DN + g2 * 2 * DV:
                                         NH * DN + (g2 + 1) * 2 * DV]
                    .rearrange("(k p) c -> p k c", p=P))
            v_sb = pdv.tile([P, KB_S, 2 * DV], BF16, tag="v")
            for kb in range(KB_S):
                psv = psKV.tile([P, W], F32, tag="pskv")
                for kc in range(KB_CKV):
                    nc.tensor.matmul(
                        psv[:, :2 * DV],
                        ckF[:, kc, kb * P:(kb + 1) * P], wv[:, kc, :],
                        start=(kc == 0), stop=(kc == KB_CKV - 1))
                nc.scalar.activation(v_sb[:, kb, :], psv[:, :2 * DV], COPY)
            for hl in range(2):
                h = g2 * 2 + hl
                knT = knt_map[h][0]
                po = psO.tile([P, W], F32, tag="po")
                prr = psR.tile([P, W], F32, tag="prr")
                plist = []
                for kt in range(NKT):
                    pss = psSc.tile([P, 2, W], F32, tag="pss")
                    for tt in range(2):
                        kb = 2 * kt + tt
                        sc, j = divmod(kb, 4)
                        nc.tensor.matmul(
                            pss[:, tt, :],
                            knT[:, sc, j * P:(j + 1) * P],
                            qnope[:, h, :], start=True, stop=False)
                        nc.tensor.matmul(
                            pss[:, tt, :],
                            kpe2f[:, kb * P:(kb + 1) * P],
                            qpe[:, h, :], start=False, stop=True)
                    probs = pprob.tile([P, 2, W], BF16, tag="probs")
                    if with_mask:
                        pmf = pd.tile([P, 2, W], F32, tag="pmf")
                        mtile = mask_pool.tile([P, 2, W], F32, tag="mt")
                        nc.sync.dma_start(
                            mtile[:], t["maskT"]
                            .rearrange("(n tp) q -> tp n q", tp=P)
                            [:, 2 * kt:2 * kt + 2, :])
                        nc.vector.scalar_tensor_tensor(
                            pmf[:], pss[:], SCALE, mtile[:], MULT, ADD)
                        nc.scalar.activation(probs[:], pmf[:], EXP)
                    else:
                        nc.scalar.activation(probs[:], pss[:], EXP,
                                             scale=SCALE)
                    plist.append(probs)
                    # next head's k_nope, one full head of lag
                    if kt % 2 == 1 and h + 1 < NH:
                        emit_kn_block(h + 1, (kt - 1) // 2, knt_map)
                    if kt >= 3:
                        _av(nc, v_sb, hl, plist[kt - 3], po, prr,
                            ones_mat, kt - 3)
                for kt in range(NKT - 3, NKT):
                    _av(nc, v_sb, hl, plist[kt], po, prr, ones_mat, kt)
                rec = pd.tile([P, W], F32, tag="rec")
                nc.vector.reciprocal_approx_fast(rec[:], prr[:])
                nc.vector.tensor_tensor(oT_sb[:, h, :], po[:], rec[:], MULT)
                del knt_map[h]
    kpe_pool.release()
    q8_pool.release()

    # ---------------- phase E: o_proj --------------------------------
    with tc.tile_pool(name="phE", bufs=2) as pe, \
         tc.tile_pool(name="phE_w", bufs=2) as pew, \
         tc.tile_pool(name="psA", bufs=2, space="PSUM") as psA:
        for m in range(MB_HID):
            wm = pew.tile([P, NH, P], BF16, tag="wo")
            nc.sync.dma_start(
                wm[:], t["w_o"][:, m * P:(m + 1) * P]
                .rearrange("(k p) c -> p k c", p=P))
            ps = psA.tile([P, W], F32, tag="psA")
            for k in range(NH):
                nc.tensor.matmul(ps[:], wm[:, k, :], oT_sb[:, k, :],
                                 start=(k == 0), stop=(k == NH - 1))
            osb = pe.tile([P, W], F32, tag="osb")
            nc.scalar.activation(osb[:], ps[:], COPY)
            nc.sync.dma_start(t["outT"][m * P:(m + 1) * P, :], osb[:])
    o_pool.release()
    kpe_pool0.release()
    qa_pool.release()
    const.release()


def _av(nc, v_sb, hl, probs, po, prr, ones_mat, kt):
    for tt in range(2):
        kb = 2 * kt + tt
        nc.tensor.matmul(po[:], v_sb[:, kb, hl * DV:(hl + 1) * DV],
                         probs[:, tt, :], start=(kb == 0),
                         stop=(kb == KB_S - 1))
        nc.tensor.matmul(prr[:], ones_mat[:], probs[:, tt, :],
                         start=(kb == 0), stop=(kb == KB_S - 1))


def _build_program(with_mask):
    nc = bacc.Bacc("TRN2", target_bir_lowering=False, debug=False,
                   num_devices=NCORES)
    t = {}

    def inp(name, shape, dt=F32):
        t[name] = nc.dram_tensor(name, list(shape), dt,
                                 kind="ExternalInput").ap()

    inp("hsT_rot", [HID, S], BF16)
    inp("w_qa", [HID, QLR], BF16)
    inp("w_qb_re", [QLR, NH * DQK], BF16)
    inp("w_kva", [HID, KVLR + DR], BF16)
    inp("w_kvb_re", [KVLR, NH * (DN + DV)], BF16)
    inp("w_o", [NH * DV, HID], BF16)
    inp("qa_ln_p", [P, KB_QLR])
    inp("kva_ln_p", [P, KB_CKV])
    inp("cos1f", [DR, S])
    inp("sin1sf", [DR, S])
    inp("cos2p", [P, W])
    inp("sin2sp", [P, W])
    if with_mask:
        inp("maskT", [S, W])
    t["outT"] = nc.dram_tensor("outT", [HID, W], F32,
                               kind="ExternalOutput").ap()

    with tile.TileContext(nc) as tc:
        with nc.allow_low_precision(reason="bf16/fp8 kernel, tol 2e-2"):
            _emit(tc, t, with_mask)
    nc.compile()
    return nc


_PROG_CACHE = {}


def _get_program(with_mask):
    if with_mask not in _PROG_CACHE:
        _PROG_CACHE[with_mask] = _build_program(with_mask)
    return _PROG_CACHE[with_mask]


def make_in_maps(hidden_states, attention_mask, cos, sin, w_qa, qa_ln, w_qb,
                 w_kva, kva_ln, w_kvb, w_o, with_mask):
    f32, bf16 = np.float32, ml_dtypes.bfloat16
    c = np.ascontiguousarray

    w_qb_r = np.asarray(w_qb).reshape(QLR, NH, DQK)
    w_qb_re = c(np.concatenate(
        [w_qb_r[:, :, :DN].reshape(QLR, NH * DN),
         w_qb_r[:, :, DN:].reshape(QLR, NH * DR)], axis=1).astype(bf16))
    w_kvb_r = np.asarray(w_kvb).reshape(KVLR, NH, DN + DV)
    w_kvb_re = c(np.concatenate(
        [w_kvb_r[:, :, :DN].reshape(KVLR, NH * DN),
         w_kvb_r[:, :, DN:].reshape(KVLR, NH * DV)], axis=1).astype(bf16))
    qa_ln_p = c(np.asarray(qa_ln).reshape(KB_QLR, P).T.astype(f32))
    kva_ln_p = c(np.asarray(kva_ln).reshape(KB_CKV, P).T.astype(f32))

    cosT = np.asarray(cos).T.astype(f32)                  # [64, S]
    sinT = np.asarray(sin).T.astype(f32)
    sin_s = np.concatenate([-sinT[:DR // 2], sinT[DR // 2:]], axis=0)
    cos2 = c(np.concatenate([cosT, cosT], axis=0))        # [128, S]
    sin2s = c(np.concatenate([sin_s, sin_s], axis=0))

    shared = {
        "w_qa": c(np.asarray(w_qa).astype(bf16)),
        "w_qb_re": w_qb_re,
        "w_kvb_re": w_kvb_re,
        "w_kva": c(np.asarray(w_kva).astype(bf16)),
        "w_o": c(np.asarray(w_o).astype(bf16)),
        "qa_ln_p": qa_ln_p,
        "kva_ln_p": kva_ln_p,
    }

    hs = np.asarray(hidden_states)
    am = np.asarray(attention_mask)
    in_maps = []
    for core in range(NCORES):
        b, q = divmod(core, NQ)
        q0 = q * W
        m = dict(shared)
        rot = np.r_[np.arange(q0, S), np.arange(0, q0)]
        m["hsT_rot"] = c(hs[b].T[:, rot].astype(bf16))
        m["cos1f"] = c(cosT[:, rot])
        m["sin1sf"] = c(sin_s[:, rot])
        m["cos2p"] = c(cos2[:, q0:q0 + W])
        m["sin2sp"] = c(sin2s[:, q0:q0 + W])
        if with_mask:
            m["maskT"] = c(am[b, 0, q0:q0 + W, :].T[rot, :].astype(f32))
        in_maps.append(m)
    return in_maps


def kernel(hidden_states, attention_mask, cos, sin, w_qa, qa_ln, w_qb,
           w_kva, kva_ln, w_kvb, w_o):
    global LAST_RESULT
    with_mask = bool(np.any(np.asarray(attention_mask) != 0))
    nc = _get_program(with_mask)
    in_maps = make_in_maps(hidden_states, attention_mask, cos, sin, w_qa,
                           qa_ln, w_qb, w_kva, kva_ln, w_kvb, w_o, with_mask)
    trace = os.environ.get("KERNEL_TRACE", "0") == "1"
    res = bass_utils.run_bass_kernel_spmd(
        nc, in_maps, core_ids=list(range(NCORES)), trace=trace)
    LAST_RESULT = res

    out = np.empty((B, S, HID), np.float32)
    for core in range(NCORES):
        b, q = divmod(core, NQ)
        q0 = q * W
        out[b, q0:q0 + W, :] = res.results[core]["outT"].T
    return out
